# revision 1
# baseline (speedup 1.0000x reference)
"""Trainium2 Bass kernel for nn_MEGNet_State_876173328941.

MEGNet state update: u_e = scatter_mean(edge_attr, batch[edge_index[0]], B),
u_v = scatter_mean(x, batch, B), comb = [u_e, u_v, state], then a 3-layer MLP
(96->32->32->32) with training-mode BatchNorm over the batch dim.

Sharding strategy (host side, inside kernel()):
  - The 1024 graphs are assigned to the 8 cores with a balanced (LPT)
    partition of their edge-tile counts; each core owns 128 graphs. Within a
    core, graphs are ranked by size; slot i's tile count (sched_e[i]) is the
    max over cores at that rank, so all cores share ONE SPMD program. Rows
    are zero-padded into their slots with a 33rd "ones" column marking real
    rows (the device computes per-graph counts itself).
  - Device: each 128-row tile is reduced with one TensorE matmul
    (lhsT = rows [128, 33], rhs = ones [128, 1]) accumulating straight into
    PSUM column i of a per-core [33, 129] segment-sum accumulator
    (column 128 is a scratch column for pad tiles).
  - Per-core partial results are AllGathered; every core then computes the
    scatter-mean division and the tiny MLP with BatchNorm redundantly in
    transposed layout [feat, graph]. Host takes core 0's output and undoes
    the graph permutation.
"""

import sys

sys.path.insert(0, "/opt/trn_rl_repo")

import numpy as np

import concourse.bacc as bacc
import concourse.tile as tile
from concourse import mybir
from concourse.bass_utils import run_bass_kernel_spmd

DIM = 32
DIMC = DIM + 1      # +1 ones column for counts
B = 1024
N_CORES = 8
SEGS = 128          # graphs per core
CH = 128            # tiles per DMA chunk
EPS = 1e-5
AGR = 128           # allgather rows: 0-31 e-sums, 32-63 v-sums, 64 e-cnt, 96 v-cnt

_CACHE = {}


def _plan(ecnt, ncnt):
    """Balanced graph->core assignment plus shared per-rank slot schedule."""
    e_tiles = np.maximum((ecnt + 127) // 128, 1).astype(np.int64)
    n_tiles = np.maximum((ncnt + 127) // 128, 1).astype(np.int64)

    order_desc = np.argsort(-e_tiles, kind="stable")
    load = np.zeros(N_CORES, dtype=np.int64)
    nseg = np.zeros(N_CORES, dtype=np.int64)
    assign = np.zeros(B, dtype=np.int64)
    for s in order_desc:
        open_cores = np.where(nseg < SEGS)[0]
        k = open_cores[np.argmin(load[open_cores])]
        assign[s] = k
        load[k] += e_tiles[s]
        nseg[k] += 1

    # per-core rank order: this core's graphs sorted by e_tiles desc
    order = np.zeros((N_CORES, SEGS), dtype=np.int64)   # rank -> global seg
    rank_of = np.zeros(B, dtype=np.int64)
    for k in range(N_CORES):
        segs_k = np.where(assign == k)[0]
        segs_k = segs_k[np.argsort(-e_tiles[segs_k], kind="stable")]
        order[k] = segs_k
        rank_of[segs_k] = np.arange(SEGS)

    sched_e = e_tiles[order].max(axis=0)   # [SEGS]
    sched_n = n_tiles[order].max(axis=0)   # [SEGS]
    p_global = order.reshape(-1)           # gathered col j -> global seg
    return assign, rank_of, sched_e, sched_n, p_global


def _tile_plan(sched):
    """[(col, start, stop)] per tile, padded to a CH multiple with scratch."""
    plan = []
    for i, t in enumerate(sched):
        for j in range(int(t)):
            plan.append((i, j == 0, j == int(t) - 1))
    while len(plan) % CH:
        plan.append((SEGS, True, True))   # scratch column
    return plan


def _build_nc(plan_e, plan_n):
    nc = bacc.Bacc("TRN2", target_bir_lowering=False, debug=False,
                   enable_asserts=False, num_devices=N_CORES)
    f32 = mybir.dt.float32

    ev_chunks = len(plan_e) // CH
    nv_chunks = len(plan_n) // CH
    ev = nc.declare_dram_parameter("ev", [ev_chunks, 128, CH * DIMC], f32, isOutput=False)
    nv = nc.declare_dram_parameter("nv", [nv_chunks, 128, CH * DIMC], f32, isOutput=False)
    stateT = nc.declare_dram_parameter("stateT", [DIM, B], f32, isOutput=False)
    W1 = nc.declare_dram_parameter("W1", [3 * DIM, DIM], f32, isOutput=False)
    W2 = nc.declare_dram_parameter("W2", [DIM, DIM], f32, isOutput=False)
    W3 = nc.declare_dram_parameter("W3", [DIM, DIM], f32, isOutput=False)
    # vecs columns: b1,g1,be1,b2,g2,be2,b3,g3,be3
    vecs = nc.declare_dram_parameter("vecs", [DIM, 9], f32, isOutput=False)
    out = nc.declare_dram_parameter("out", [DIM, B], f32, isOutput=True)

    ag_in = nc.dram_tensor("ag_in", [AGR, SEGS], f32)
    ag_out = nc.dram_tensor("ag_out", [AGR * N_CORES, SEGS], f32,
                            addr_space="Shared")

    with tile.TileContext(nc) as tc:
        with tc.tile_pool(name="chunks", bufs=3) as chunks, \
             tc.tile_pool(name="const", bufs=1) as const, \
             tc.tile_pool(name="work", bufs=1) as work, \
             tc.tile_pool(name="spsum", bufs=1, space="PSUM") as spsum, \
             tc.tile_pool(name="mpsum", bufs=1, space="PSUM") as mpsum:

            ones = const.tile([128, 1], f32)
            nc.vector.memset(ones, 1.0)
            onesP = const.tile([128, DIM], f32)
            nc.vector.memset(onesP, 1.0)

            # ---- stage 1: streamed per-graph segment sums ----
            ps_e = spsum.tile([DIMC, SEGS + 1], f32, tag="ps_e")
            ps_n = spsum.tile([DIMC, SEGS + 1], f32, tag="ps_n")

            def stream(param, plan, psum_tile):
                n_chunks = len(plan) // CH
                for c in range(n_chunks):
                    ct = chunks.tile([128, CH * DIMC], f32, tag="chunk")
                    nc.sync.dma_start(out=ct, in_=param[c])
                    for t in range(CH):
                        col, start, stop = plan[c * CH + t]
                        nc.tensor.matmul(
                            out=psum_tile[:, col:col + 1],
                            lhsT=ct[:, t * DIMC:(t + 1) * DIMC],
                            rhs=ones[:, :],
                            start=start,
                            stop=stop,
                        )

            stream(ev, plan_e, ps_e)
            stream(nv, plan_n, ps_n)

            sums_e = work.tile([DIMC, SEGS], f32, tag="sums_e")
            nc.vector.tensor_copy(sums_e, ps_e[:, 0:SEGS])
            sums_n = work.tile([DIMC, SEGS], f32, tag="sums_n")
            nc.vector.tensor_copy(sums_n, ps_n[:, 0:SEGS])

            # ---- collective: gather all cores' slices ----
            zrows = const.tile([128, SEGS], f32)
            nc.vector.memset(zrows, 0.0)
            nc.sync.dma_start(out=ag_in[:, :], in_=zrows)
            nc.sync.dma_start(out=ag_in[0:DIM, :], in_=sums_e[0:DIM, :])
            nc.sync.dma_start(out=ag_in[DIM:2 * DIM, :], in_=sums_n[0:DIM, :])
            nc.sync.dma_start(out=ag_in[64:65, :], in_=sums_e[DIM:DIMC, :])
            nc.sync.dma_start(out=ag_in[96:97, :], in_=sums_n[DIM:DIMC, :])
            nc.gpsimd.collective_compute(
                "AllGather",
                mybir.AluOpType.bypass,
                replica_groups=[list(range(N_CORES))],
                ins=[ag_in[:, :]],
                outs=[ag_out[:, :]],
            )
            full = work.tile([AGR, B], f32, tag="full")
            agv = ag_out.rearrange("(r p) s -> r p s", p=AGR)
            for r in range(N_CORES):
                nc.sync.dma_start(out=full[:, r * SEGS:(r + 1) * SEGS], in_=agv[r])

            # ---- scatter-mean division ----
            rec = work.tile([AGR, B], f32, tag="rec")
            nc.vector.tensor_scalar_max(rec[64:97, :], full[64:97, :], 1.0)
            nc.vector.reciprocal(rec[64:97, :], rec[64:97, :])

            # broadcast recip rows across DIM partitions via matmul
            pb = mpsum.tile([2 * DIM, B], f32, tag="pb")
            for half in range(2):
                sl = slice(half * 512, (half + 1) * 512)
                nc.tensor.matmul(out=pb[0:DIM, sl], lhsT=onesP[64:65, :],
                                 rhs=rec[64:65, sl], start=True, stop=True,
                                 tile_position=(64, 0))
                nc.tensor.matmul(out=pb[DIM:2 * DIM, sl], lhsT=onesP[96:97, :],
                                 rhs=rec[96:97, sl], start=True, stop=True,
                                 tile_position=(96, 32))

            comb = work.tile([3 * DIM, B], f32, tag="comb")
            nc.vector.tensor_tensor(comb[0:DIM, :], full[0:DIM, :],
                                    pb[0:DIM, :], mybir.AluOpType.mult)
            nc.vector.tensor_tensor(comb[DIM:2 * DIM, :], full[DIM:2 * DIM, :],
                                    pb[DIM:2 * DIM, :], mybir.AluOpType.mult)
            nc.sync.dma_start(out=comb[2 * DIM:3 * DIM, :], in_=stateT[:, :])

            # ---- MLP with BatchNorm (transposed layout [feat, graph]) ----
            w1s = const.tile([3 * DIM, DIM], f32)
            nc.sync.dma_start(out=w1s, in_=W1[:, :])
            w2s = const.tile([DIM, DIM], f32)
            nc.sync.dma_start(out=w2s, in_=W2[:, :])
            w3s = const.tile([DIM, DIM], f32)
            nc.sync.dma_start(out=w3s, in_=W3[:, :])
            vs = const.tile([DIM, 9], f32)
            nc.sync.dma_start(out=vs, in_=vecs[:, :])

            h = comb
            for layer in range(3):
                w = (w1s, w2s, w3s)[layer]
                bcol = vs[:, 3 * layer:3 * layer + 1]
                gcol = vs[:, 3 * layer + 1:3 * layer + 2]
                becol = vs[:, 3 * layer + 2:3 * layer + 3]

                ps_h = mpsum.tile([DIM, B], f32, tag="ps_h")
                for half in range(2):
                    sl = slice(half * 512, (half + 1) * 512)
                    nc.tensor.matmul(out=ps_h[:, sl], lhsT=w[:, :], rhs=h[:, sl],
                                     start=True, stop=True)
                hl = work.tile([DIM, B], f32, tag=f"h{layer}")
                func = (mybir.ActivationFunctionType.Relu if layer < 2
                        else mybir.ActivationFunctionType.Identity)
                nc.scalar.activation(out=hl, in_=ps_h, func=func, bias=bcol)

                # batchnorm over the free (graph) dim
                msum = work.tile([DIM, 1], f32, tag="msum")
                nc.vector.tensor_reduce(out=msum, in_=hl,
                                        axis=mybir.AxisListType.X,
                                        op=mybir.AluOpType.add)
                m = work.tile([DIM, 1], f32, tag="m")
                nc.scalar.mul(m, msum, 1.0 / B)
                hc = work.tile([DIM, B], f32, tag=f"hc{layer}")
                nc.vector.tensor_scalar(hc, hl, m, None,
                                        mybir.AluOpType.subtract)
                sq = work.tile([DIM, B], f32, tag="sq")
                vsum = work.tile([DIM, 1], f32, tag="vsum")
                nc.scalar.activation(out=sq, in_=hc,
                                     func=mybir.ActivationFunctionType.Square,
                                     accum_out=vsum)
                veps = work.tile([DIM, 1], f32, tag="veps")
                nc.scalar.activation(out=veps, in_=vsum,
                                     func=mybir.ActivationFunctionType.Copy,
                                     bias=EPS, scale=1.0 / B)
                sd = work.tile([DIM, 1], f32, tag="sd")
                nc.scalar.sqrt(sd, veps)
                rstd = work.tile([DIM, 1], f32, tag="rstd")
                nc.vector.reciprocal(rstd, sd)
                rg = work.tile([DIM, 1], f32, tag="rg")
                nc.vector.tensor_tensor(rg, rstd, gcol, mybir.AluOpType.mult)
                hb = work.tile([DIM, B], f32, tag=f"hb{layer}")
                nc.vector.tensor_scalar(hb, hc, rg, becol,
                                        mybir.AluOpType.mult,
                                        mybir.AluOpType.add)
                h = hb

            nc.sync.dma_start(out=out[:, :], in_=h)

    nc.compile()
    return nc


def _pack(rows, seg, cnt, assign, rank_of, sched):
    """Scatter rows (f32 [M, 33], ones col included) into per-core DMA layout
    [N_CORES, n_chunks, 128, CH*33] per the shared slot schedule."""
    M = rows.shape[0]
    base = np.zeros(SEGS + 1, dtype=np.int64)
    np.cumsum(sched, out=base[1:])            # slot base tile per rank
    total_tiles = int(base[-1])
    n_chunks = (total_tiles + CH - 1) // CH
    pad_tiles = n_chunks * CH

    order = np.argsort(seg, kind="stable")
    srows = rows[order]
    sseg = seg[order]
    offs = np.zeros(B, dtype=np.int64)
    np.cumsum(cnt[:-1], out=offs[1:])
    within = np.arange(M, dtype=np.int64) - offs[sseg]

    core = assign[sseg]
    rank = rank_of[sseg]
    g = base[rank] + (within >> 7)            # tile within core
    c, t, p = g // CH, g % CH, within & 127
    P = np.zeros((N_CORES, n_chunks, 128, CH, DIMC), dtype=np.float32)
    P[core, c, p, t] = srows
    return P.reshape(N_CORES, n_chunks, 128, CH * DIMC)


def run(inputs, trace=False, sim=False):
    x = np.asarray(inputs["x"], dtype=np.float32)
    edge_index = np.asarray(inputs["edge_index"]).astype(np.int64)
    edge_attr = np.asarray(inputs["edge_attr"], dtype=np.float32)
    state = np.asarray(inputs["state"], dtype=np.float32)
    batch = np.asarray(inputs["batch"]).astype(np.int64)

    E = edge_attr.shape[0]
    N = x.shape[0]
    eseg = batch[edge_index[0]]
    ecnt = np.bincount(eseg, minlength=B)
    ncnt = np.bincount(batch, minlength=B)

    assign, rank_of, sched_e, sched_n, p_global = _plan(ecnt, ncnt)
    plan_e = _tile_plan(sched_e)
    plan_n = _tile_plan(sched_n)

    erows = np.empty((E, DIMC), dtype=np.float32)
    erows[:, :DIM] = edge_attr
    erows[:, DIM] = 1.0
    nrows = np.empty((N, DIMC), dtype=np.float32)
    nrows[:, :DIM] = x
    nrows[:, DIM] = 1.0

    ev = _pack(erows, eseg, ecnt, assign, rank_of, sched_e)
    nv = _pack(nrows, batch, ncnt, assign, rank_of, sched_n)

    vecs = np.stack([np.asarray(inputs[k], np.float32) for k in
                     ("b1", "g1", "be1", "b2", "g2", "be2", "b3", "g3", "be3")],
                    axis=1).astype(np.float32)  # [32, 9]

    shared = {
        "stateT": np.ascontiguousarray(state.T[:, p_global]),
        "W1": np.asarray(inputs["W1"], np.float32),
        "W2": np.asarray(inputs["W2"], np.float32),
        "W3": np.asarray(inputs["W3"], np.float32),
        "vecs": vecs,
    }
    in_maps = []
    for k in range(N_CORES):
        m = dict(shared)
        m["ev"] = np.ascontiguousarray(ev[k])
        m["nv"] = np.ascontiguousarray(nv[k])
        in_maps.append(m)

    key = (tuple(sched_e), tuple(sched_n))
    if key not in _CACHE:
        _CACHE[key] = _build_nc(plan_e, plan_n)
    nc = _CACHE[key]

    if sim:
        from concourse.bass_interp import MultiCoreSim
        msim = MultiCoreSim(nc, num_cores=N_CORES)
        for c in range(N_CORES):
            cs = msim.cores[c]
            for kk, vv in in_maps[c].items():
                cs.tensor(kk)[:] = vv
        msim.simulate(check_with_hw=False)
        outT = np.array(msim.cores[0].tensor("out"))
        res = None
    else:
        res = run_bass_kernel_spmd(nc, in_maps, core_ids=list(range(N_CORES)),
                                   trace=trace)
        outT = res.results[0]["out"]  # [32, 1024] in permuted graph order

    outP = outT.T.astype(np.float32)          # [1024(perm), 32]
    outF = np.empty_like(outP)
    outF[p_global] = outP
    return np.ascontiguousarray(outF), res


def kernel(**inputs) -> np.ndarray:
    out, _ = run(inputs, trace=False)
    return out



# revision 2
# speedup vs baseline: 1.1359x; 1.1359x over previous
"""Trainium2 Bass kernel for nn_MEGNet_State_876173328941.

MEGNet state update: u_e = scatter_mean(edge_attr, batch[edge_index[0]], B),
u_v = scatter_mean(x, batch, B), comb = [u_e, u_v, state], then a 3-layer MLP
(96->32->32->32) with training-mode BatchNorm over the batch dim.

v2.1 strategy:
  - Host: LPT-balance the 1024 graphs over 8 cores (128 slots each, shared
    per-rank tile schedule so one SPMD program serves all cores). Edge/node
    rows are quantized to fp8e3m4 (4-bit mantissa; |x|<=15.5 covers N(0,1))
    and packed into 128-row tiles, 32 features per tile, slot-major.
  - Device stage 1: per slot the tiles form <=16-tile groups. Group j uses a
    one-hot-column stationary operand (a [128, 32] window of a constant
    [128, 64] "mega" tile, ones in window-column k = j mod 32) and streams
    its tiles as the moving rhs [128, 32*len]; the column sums land on PSUM
    partition row k of region j//32 (regions are [32, 512] PSUM banks,
    initialized by one zero-matmul so every element has a defined
    accumulation start). PE runs at its ~1 col/cycle streaming rate.
  - Drain: per region one strided DVE reduce folds the 16 tile-positions ->
    stash[32, 32] (32-partition parallel). A host-built membership matmul
    (mem[k, slot]) then accumulates group sums into slotsums [128 slots, 64]
    (edge cols 0:32, node cols 32:64).
  - Host-exact reciprocal counts turn sums into means, one PE transpose
    yields the [64, 128] AllGather slice; every core then runs the tiny MLP
    redundantly in [feat, graph] layout, bf16 activations, f32 stats.
"""

import sys

sys.path.insert(0, "/opt/trn_rl_repo")

import numpy as np
import ml_dtypes

import concourse.bacc as bacc
import concourse.tile as tile
from concourse import mybir
from concourse.bass_utils import run_bass_kernel_spmd

DIM = 32
B = 1024
N_CORES = 8
SEGS = 128          # graph slots per core
GRP = 16            # node: max tiles per matmul group (rhs free <= 512)
GRP_E = 32          # edge: tiles per DoubleRow group (256-row contraction)
KROT = 32           # node: groups per PSUM region (partition rows)
KROT_E = 32         # edge: groups per PSUM region
CH = 512            # tiles per DMA chunk (2 MiB fp8)
EPS = 1e-5

F8 = ml_dtypes.float8_e3m4        # nodes (e3m4: best precision for N(0,1))
F8E = ml_dtypes.float8_e4m3       # edges (e4m3: required for DoubleRow)

_CACHE = {}


def _plan(ecnt, ncnt):
    """Balanced graph->core assignment plus shared per-rank slot schedule."""
    e_tiles = np.maximum((ecnt + 127) // 128, 1).astype(np.int64)
    n_tiles = np.maximum((ncnt + 127) // 128, 1).astype(np.int64)

    order_desc = np.argsort(-e_tiles, kind="stable")
    load = np.zeros(N_CORES, dtype=np.int64)
    nseg = np.zeros(N_CORES, dtype=np.int64)
    assign = np.zeros(B, dtype=np.int64)
    for s in order_desc:
        open_cores = np.where(nseg < SEGS)[0]
        k = open_cores[np.argmin(load[open_cores])]
        assign[s] = k
        load[k] += e_tiles[s]
        nseg[k] += 1

    order = np.zeros((N_CORES, SEGS), dtype=np.int64)   # rank -> global seg
    rank_of = np.zeros(B, dtype=np.int64)
    for k in range(N_CORES):
        segs_k = np.where(assign == k)[0]
        segs_k = segs_k[np.argsort(-e_tiles[segs_k], kind="stable")]
        order[k] = segs_k
        rank_of[segs_k] = np.arange(SEGS)

    sched_e = e_tiles[order].max(axis=0)   # [SEGS]
    sched_e = sched_e + (sched_e & 1)      # even (DoubleRow pairs rows)
    sched_n = n_tiles[order].max(axis=0)   # [SEGS]
    p_global = order.reshape(-1)           # gathered col j -> global seg
    return assign, rank_of, sched_e, sched_n, p_global


def _qplan(sched, grp):
    """Groups: (slot, global_start_tile, n_tiles), slot-major stream order.

    Groups never span a slot boundary, a chunk boundary, or exceed grp
    tiles, so each group is a single self-contained matmul.
    """
    groups = []
    t0 = 0
    for slot, t in enumerate(sched):
        t = int(t)
        off = 0
        while off < t:
            gs = t0 + off
            L = min(grp, t - off, CH - gs % CH)
            groups.append((slot, gs, L))
            off += L
        t0 += t
    return groups, t0


def _mm_plan(groups, T, krot):
    """Per-chunk matmuls + per-region membership info.

    chunk_mms[c] entries in program order:
      ('zero', region)               -- region-init zero matmul
      ('mm', a, n, region, k, stop)  -- one group (psum cols 0:colw*n)
    mems[r][k] = slot owning group KROT*r+k (or -1).
    """
    n_chunks = (T + CH - 1) // CH
    chunk_mms = [[] for _ in range(n_chunks)]
    NR = (len(groups) + krot - 1) // krot
    mems = -np.ones((NR, KROT), dtype=np.int64)
    last_mm = {}             # region -> (chunk, idx)
    for j, (slot, gs, L) in enumerate(groups):
        r, k = j // krot, j % krot
        mems[r, k] = slot
        c = gs // CH
        if k == 0:
            chunk_mms[c].append(("zero", r))
        chunk_mms[c].append(("mm", gs - c * CH, L, r, k, False))
        last_mm[r] = (c, len(chunk_mms[c]) - 1)
    for r, (c, i) in last_mm.items():
        e = chunk_mms[c][i]
        chunk_mms[c][i] = e[:-1] + (True,)
    return chunk_mms, NR, mems


def _build_nc(sched_e, sched_n):
    groups_e, T_e = _qplan(sched_e, GRP_E)
    groups_n, T_n = _qplan(sched_n, GRP)
    mm_e, NR_e, mems_e = _mm_plan(groups_e, T_e, KROT_E)
    mm_n, NR_n, mems_n = _mm_plan(groups_n, T_n, KROT)
    NR = NR_e + NR_n

    nc = bacc.Bacc("TRN2", target_bir_lowering=False, debug=False,
                   enable_asserts=False, num_devices=N_CORES)
    f32 = mybir.dt.float32
    bf16 = mybir.dt.bfloat16
    f8 = mybir.dt.float8e3
    f8e = mybir.dt.float8e4

    ev = nc.declare_dram_parameter("ev", [128, T_e * DIM], f8e, isOutput=False)
    nv = nc.declare_dram_parameter("nv", [128, T_n * DIM], f8, isOutput=False)
    # megaE cols (i, c): ones at c=KROT for both i halves (DoubleRow lhsT)
    megaEP = nc.declare_dram_parameter("megaEP", [128, 4 * KROT], f8e,
                                       isOutput=False)
    megaP = nc.declare_dram_parameter("megaP", [128, 2 * KROT], f8, isOutput=False)
    memsP = nc.declare_dram_parameter("memsP", [KROT, NR * SEGS], f32, isOutput=False)
    recL = nc.declare_dram_parameter("recL", [SEGS, 2], f32, isOutput=False)
    ident = nc.declare_dram_parameter("ident", [128, 128], f32, isOutput=False)
    stateT = nc.declare_dram_parameter("stateT", [DIM, B], bf16, isOutput=False)
    W1 = nc.declare_dram_parameter("W1", [3 * DIM, DIM], bf16, isOutput=False)
    W2 = nc.declare_dram_parameter("W2", [DIM, DIM], bf16, isOutput=False)
    W3 = nc.declare_dram_parameter("W3", [DIM, DIM], bf16, isOutput=False)
    # vecs columns: b1,g1,be1,b2,g2,be2,b3,g3,be3
    vecs = nc.declare_dram_parameter("vecs", [DIM, 9], f32, isOutput=False)
    out = nc.declare_dram_parameter("out", [DIM, B], f32, isOutput=True)

    AGR = 64            # allgather rows: 0-31 e-means, 32-63 v-means
    ag_in = nc.dram_tensor("ag_in", [AGR, SEGS], f32)
    ag_out = nc.dram_tensor("ag_out", [AGR * N_CORES, SEGS], f32,
                            addr_space="Shared")

    with tile.TileContext(nc) as tc:
        with tc.tile_pool(name="chunks", bufs=3) as chunks, \
             tc.tile_pool(name="const", bufs=1) as const, \
             tc.tile_pool(name="work", bufs=1) as work, \
             tc.tile_pool(name="qpsum", bufs=4, space="PSUM") as qpsum, \
             tc.tile_pool(name="spsum", bufs=1, space="PSUM") as spsum, \
             tc.tile_pool(name="mpsum", bufs=1, space="PSUM") as mpsum:

            # constants (issued first so DMA overlaps the edge stream)
            megaE = const.tile([128, 4 * KROT], f8e)
            nc.sync.dma_start(out=megaE, in_=megaEP[:, :])
            mega = const.tile([128, 2 * KROT], f8)
            nc.sync.dma_start(out=mega, in_=megaP[:, :])
            memsT = const.tile([KROT, NR * SEGS], f32)
            nc.sync.dma_start(out=memsT, in_=memsP[:, :])
            rec = const.tile([SEGS, 2], f32)
            nc.sync.dma_start(out=rec, in_=recL[:, :])
            idn = const.tile([128, 128], f32)
            nc.sync.dma_start(out=idn, in_=ident[:, :])
            w1s = const.tile([3 * DIM, DIM], bf16)
            nc.sync.dma_start(out=w1s, in_=W1[:, :])
            w2s = const.tile([DIM, DIM], bf16)
            nc.sync.dma_start(out=w2s, in_=W2[:, :])
            w3s = const.tile([DIM, DIM], bf16)
            nc.sync.dma_start(out=w3s, in_=W3[:, :])
            vs = const.tile([DIM, 9], f32)
            nc.sync.dma_start(out=vs, in_=vecs[:, :])
            zerot = const.tile([128, GRP * DIM], f8)
            nc.vector.memset(zerot, 0.0)
            zerotE = const.tile([128, GRP * DIM], f8e)
            nc.vector.memset(zerotE, 0.0)
            sq_warm = work.tile([DIM, 1], f32, tag="sq_warm")
            nc.scalar.activation(out=sq_warm, in_=rec[0:DIM, 0:1],
                                 func=mybir.ActivationFunctionType.Square)
            nc.scalar.activation(out=sq_warm, in_=rec[0:DIM, 0:1],
                                 func=mybir.ActivationFunctionType.Sqrt)
            comb = work.tile([3 * DIM, B], bf16, tag="comb")
            nc.sync.dma_start(out=comb[2 * DIM:3 * DIM, :], in_=stateT[:, :])

            # ---- stage 1: streamed group sums spread over PSUM rows ----
            stash = work.tile([KROT, NR * DIM], f32, tag="stash")
            slotsums = spsum.tile([SEGS, 2 * DIM], f32, tag="slotsums")
            regs = {}

            # PE pre-warm: keep the tensor engine busy through the start
            # barrier + first-chunk DMA so the DVFS ramp starts early.
            wreg = qpsum.tile([KROT_E, GRP * DIM], mybir.dt.float32,
                              tag="reg")
            for _ in range(24):
                nc.tensor.matmul(out=wreg[:, :], lhsT=megaE[:, 0:KROT_E],
                                 rhs=zerotE[:, :], start=True, stop=True)

            def drain(q, reg, kr):
                nc.vector.tensor_reduce(
                    out=stash[0:kr, DIM * q:DIM * (q + 1)],
                    in_=reg[:, :].rearrange("p (t f) -> p f t", f=DIM),
                    axis=mybir.AxisListType.X, op=mybir.AluOpType.add)

            def memmm(q, cc, start, stop, kr):
                nc.tensor.matmul(
                    out=slotsums[:, cc:cc + DIM],
                    lhsT=memsT[0:kr, SEGS * q:SEGS * (q + 1)],
                    rhs=stash[0:kr, DIM * q:DIM * (q + 1)],
                    start=start, stop=stop)

            def stream(param, chunk_mms, T, NRS, q0, cc, dr):
                cdt = f8e if dr else f8
                ztile = zerotE if dr else zerot
                zw = megaE[:, 0:KROT] if dr else mega[:, KROT:2 * KROT]
                kr = KROT_E if dr else KROT
                n_chunks = (T + CH - 1) // CH
                for c in range(n_chunks):
                    w = min(CH, T - c * CH)
                    ct = chunks.tile([128, CH * DIM], cdt, tag="chunk")
                    nc.sync.dma_start(out=ct[:, 0:w * DIM],
                                      in_=param[:, c * CH * DIM:(c * CH + w) * DIM])
                    for e in chunk_mms[c]:
                        if e[0] == "zero":
                            r = e[1]
                            reg = qpsum.tile([kr, GRP * DIM],
                                             mybir.dt.float32, tag="reg")
                            regs[q0 + r] = reg
                            nc.tensor.matmul(
                                out=reg[:, :], lhsT=zw[:, 0:kr],
                                rhs=ztile[:, :], start=True, stop=False)
                        else:
                            _, a, n, r, k, stop = e
                            reg = regs[q0 + r]
                            if dr:
                                lw = megaE.rearrange(
                                    "p (i c) -> p i c", i=2)[
                                    :, :, KROT - k:KROT + KROT_E - k]
                                rhsap = ct[:, a * DIM:(a + n) * DIM].rearrange(
                                    "p (i q) -> p i q", i=2)
                                nc.tensor.matmul(
                                    out=reg[:, 0:(DIM // 2) * n],
                                    lhsT=lw, rhs=rhsap,
                                    start=False, stop=stop,
                                    perf_mode=mybir.MatmulPerfMode.DoubleRow)
                            else:
                                nc.tensor.matmul(
                                    out=reg[:, 0:DIM * n],
                                    lhsT=mega[:, KROT - k:2 * KROT - k],
                                    rhs=ct[:, a * DIM:(a + n) * DIM],
                                    start=False, stop=stop)
                            if stop:
                                q = q0 + r
                                drain(q, reg, kr)
                                memmm(q, cc, r == 0, r == NRS - 1, kr)

            stream(ev, mm_e, T_e, NR_e, 0, 0, True)
            stream(nv, mm_n, T_n, NR_n, NR_e, DIM, False)

            # sums -> means with host-exact reciprocal counts (bf16 out)
            sums = work.tile([SEGS, 2 * DIM], f32, tag="sums")
            nc.vector.tensor_copy(sums, slotsums)
            scaled = work.tile([SEGS, 2 * DIM], f32, tag="scaled")
            nc.vector.tensor_scalar(scaled[:, 0:DIM], sums[:, 0:DIM],
                                    rec[:, 0:1], None, mybir.AluOpType.mult)
            nc.vector.tensor_scalar(scaled[:, DIM:2 * DIM], sums[:, DIM:2 * DIM],
                                    rec[:, 1:2], None, mybir.AluOpType.mult)

            # transpose [128 slots, 64 feat] -> [64 feat, 128 slots]
            tps = spsum.tile([2 * DIM, SEGS], f32, tag="tps")
            nc.tensor.transpose(out=tps[:, :], in_=scaled[:, :], identity=idn)
            agst = work.tile([AGR, SEGS], f32, tag="agst")
            nc.vector.tensor_copy(agst, tps)
            nc.sync.dma_start(out=ag_in[:, :], in_=agst)

            # ---- collective: gather all cores' mean slices ----
            nc.gpsimd.collective_compute(
                "AllGather",
                mybir.AluOpType.bypass,
                replica_groups=[list(range(N_CORES))],
                ins=[ag_in[:, :]],
                outs=[ag_out[:, :]],
            )
            full = work.tile([AGR, B], f32, tag="full")
            nc.sync.dma_start(out=full.rearrange("p (r s) -> p r s", r=N_CORES),
                              in_=ag_out.rearrange("(r p) s -> p r s", p=AGR))
            nc.vector.tensor_copy(comb[0:2 * DIM, :], full)

            # ---- MLP with BatchNorm ([feat, graph] layout, bf16 h) ----
            h = comb
            for layer in range(3):
                w = (w1s, w2s, w3s)[layer]
                bcol = vs[:, 3 * layer:3 * layer + 1]
                gcol = vs[:, 3 * layer + 1:3 * layer + 2]
                becol = vs[:, 3 * layer + 2:3 * layer + 3]

                ps_h = mpsum.tile([DIM, B], mybir.dt.float32, tag="ps_h")
                hl = work.tile([DIM, B], bf16, tag=f"h{layer}")
                func = (mybir.ActivationFunctionType.Relu if layer < 2
                        else mybir.ActivationFunctionType.Identity)
                sh2 = work.tile([DIM, 2], f32, tag="sh2")
                for half in range(2):
                    sl = slice(half * 512, (half + 1) * 512)
                    nc.tensor.matmul(out=ps_h[:, sl], lhsT=w[:, :], rhs=h[:, sl],
                                     start=True, stop=True)
                    nc.scalar.activation(out=hl[:, sl], in_=ps_h[:, sl],
                                         func=func, bias=bcol,
                                         accum_out=sh2[:, half:half + 1])
                junk = work.tile([DIM, B], bf16, tag="junk")
                sq2 = work.tile([DIM, 2], f32, tag="sq2")
                for half in range(2):
                    sl = slice(half * 512, (half + 1) * 512)
                    nc.scalar.activation(out=junk[:, sl], in_=hl[:, sl],
                                         func=mybir.ActivationFunctionType.Square,
                                         accum_out=sq2[:, half:half + 1])

                sh = work.tile([DIM, 1], f32, tag="sh")
                nc.vector.tensor_reduce(out=sh, in_=sh2,
                                        axis=mybir.AxisListType.X,
                                        op=mybir.AluOpType.add)
                sq = work.tile([DIM, 1], f32, tag="sq")
                nc.vector.tensor_reduce(out=sq, in_=sq2,
                                        axis=mybir.AxisListType.X,
                                        op=mybir.AluOpType.add)
                m = work.tile([DIM, 1], f32, tag="m")
                nc.vector.tensor_scalar(m, sh, 1.0 / B, None,
                                        mybir.AluOpType.mult)
                msqe = work.tile([DIM, 1], f32, tag="msqe")
                nc.vector.tensor_scalar(msqe, m, m, EPS,
                                        mybir.AluOpType.mult,
                                        mybir.AluOpType.subtract)
                vpe = work.tile([DIM, 1], f32, tag="vpe")
                nc.vector.tensor_scalar(vpe, sq, 1.0 / B, msqe,
                                        mybir.AluOpType.mult,
                                        mybir.AluOpType.subtract)
                sd = work.tile([DIM, 1], f32, tag="sd")
                nc.scalar.activation(out=sd, in_=vpe,
                                     func=mybir.ActivationFunctionType.Sqrt)
                rstd = work.tile([DIM, 1], f32, tag="rstd")
                nc.vector.reciprocal(rstd, sd)
                rg = work.tile([DIM, 1], f32, tag="rg")
                nc.vector.tensor_tensor(rg, rstd, gcol, mybir.AluOpType.mult)
                mt = work.tile([DIM, 1], f32, tag="mt")
                nc.vector.tensor_tensor(mt, m, rg, mybir.AluOpType.mult)
                tt = work.tile([DIM, 1], f32, tag="tt")
                nc.vector.tensor_tensor(tt, becol, mt, mybir.AluOpType.subtract)

                if layer < 2:
                    hb = work.tile([DIM, B], bf16, tag=f"hb{layer}")
                else:
                    hb = work.tile([DIM, B], f32, tag="hb2")
                nc.vector.tensor_scalar(hb, hl, rg, tt,
                                        mybir.AluOpType.mult,
                                        mybir.AluOpType.add)
                h = hb

            nc.sync.dma_start(out=out[:, :], in_=h)

    nc.compile()
    return nc, T_e, T_n, NR_e, NR_n, mems_e, mems_n


def _pack(rows_q, seg, cnt, assign, rank_of, sched, T, groups=None):
    """Scatter fp8 rows [M, 32] into the per-core slot-major tile stream
    [N_CORES, 128, T*32] per the shared slot schedule.

    With groups (DoubleRow), each group's tiles are stored evens-then-odds
    so the matmul rhs is a clean [128, 2, 16*L] AP."""
    M = rows_q.shape[0]
    base = np.zeros(SEGS + 1, dtype=np.int64)
    np.cumsum(sched, out=base[1:])            # slot base tile per rank
    assert int(base[-1]) == T

    remap = np.arange(T, dtype=np.int64)
    if groups is not None:
        for _, gs, L in groups:
            tau = np.arange(L, dtype=np.int64)
            remap[gs:gs + L] = gs + (tau & 1) * (L // 2) + (tau >> 1)

    order = np.argsort(seg, kind="stable")
    srows = rows_q[order]
    sseg = seg[order]
    offs = np.zeros(B, dtype=np.int64)
    np.cumsum(cnt[:-1], out=offs[1:])
    within = np.arange(M, dtype=np.int64) - offs[sseg]

    core = assign[sseg]
    rank = rank_of[sseg]
    t = remap[base[rank] + (within >> 7)]     # tile within core stream
    p = within & 127
    P = np.zeros((N_CORES, 128, T, DIM), dtype=rows_q.dtype)
    P[core, p, t] = srows
    return P.reshape(N_CORES, 128, T * DIM)


def _mems_param(mems_e, mems_n):
    NR = mems_e.shape[0] + mems_n.shape[0]
    mem = np.zeros((NR, KROT, SEGS), dtype=np.float32)
    for q, row in enumerate(np.concatenate([mems_e, mems_n], axis=0)):
        for k, slot in enumerate(row):
            if slot >= 0:
                mem[q, k, slot] = 1.0
    return np.ascontiguousarray(mem.transpose(1, 0, 2).reshape(KROT, NR * SEGS))


def run(inputs, trace=False, sim=False):
    x = np.asarray(inputs["x"], dtype=np.float32)
    edge_index = np.asarray(inputs["edge_index"]).astype(np.int64)
    edge_attr = np.asarray(inputs["edge_attr"], dtype=np.float32)
    state = np.asarray(inputs["state"], dtype=np.float32)
    batch = np.asarray(inputs["batch"]).astype(np.int64)

    eseg = batch[edge_index[0]]
    ecnt = np.bincount(eseg, minlength=B)
    ncnt = np.bincount(batch, minlength=B)

    assign, rank_of, sched_e, sched_n, p_global = _plan(ecnt, ncnt)

    key = (tuple(sched_e), tuple(sched_n))
    if key not in _CACHE:
        _CACHE[key] = _build_nc(sched_e, sched_n)
    nc, T_e, T_n, NR_e, NR_n, mems_e, mems_n = _CACHE[key]

    groups_e, T_e2 = _qplan(sched_e, GRP_E)
    assert T_e2 == T_e
    ev = _pack(edge_attr.astype(F8E), eseg, ecnt, assign, rank_of, sched_e,
               T_e, groups=groups_e)
    nv = _pack(x.astype(F8), batch, ncnt, assign, rank_of, sched_n, T_n)

    rec_e = 1.0 / np.maximum(ecnt, 1).astype(np.float64)
    rec_v = 1.0 / np.maximum(ncnt, 1).astype(np.float64)
    recLf = np.stack([rec_e, rec_v], axis=1).astype(np.float32)  # [B, 2]
    recLf = recLf[p_global].reshape(N_CORES, SEGS, 2)

    mega = np.zeros((128, 2 * KROT), dtype=F8)
    mega[:, KROT] = 1.0
    megaE = np.zeros((128, 4 * KROT), dtype=F8E)
    megaE[:, KROT] = 1.0          # i=0 half: ones at window col c=KROT
    megaE[:, 3 * KROT] = 1.0      # i=1 half

    vecs = np.stack([np.asarray(inputs[k], np.float32) for k in
                     ("b1", "g1", "be1", "b2", "g2", "be2", "b3", "g3", "be3")],
                    axis=1).astype(np.float32)  # [32, 9]

    shared = {
        "megaEP": megaE,
        "megaP": mega,
        "memsP": _mems_param(mems_e, mems_n),
        "ident": np.eye(128, dtype=np.float32),
        "stateT": np.ascontiguousarray(state.T[:, p_global]).astype(ml_dtypes.bfloat16),
        "W1": np.asarray(inputs["W1"], np.float32).astype(ml_dtypes.bfloat16),
        "W2": np.asarray(inputs["W2"], np.float32).astype(ml_dtypes.bfloat16),
        "W3": np.asarray(inputs["W3"], np.float32).astype(ml_dtypes.bfloat16),
        "vecs": vecs,
    }
    in_maps = []
    for k in range(N_CORES):
        m = dict(shared)
        m["ev"] = np.ascontiguousarray(ev[k])
        m["nv"] = np.ascontiguousarray(nv[k])
        m["recL"] = np.ascontiguousarray(recLf[k])
        in_maps.append(m)

    if sim:
        from concourse.bass_interp import MultiCoreSim
        msim = MultiCoreSim(nc, num_cores=N_CORES)
        for c in range(N_CORES):
            cs = msim.cores[c]
            for kk, vv in in_maps[c].items():
                cs.tensor(kk)[:] = vv
        msim.simulate(check_with_hw=False)
        outT = np.array(msim.cores[0].tensor("out"))
        res = None
    else:
        res = run_bass_kernel_spmd(nc, in_maps, core_ids=list(range(N_CORES)),
                                   trace=trace)
        outT = res.results[0]["out"]  # [32, 1024] in permuted graph order

    outP = outT.T.astype(np.float32)          # [1024(perm), 32]
    outF = np.empty_like(outP)
    outF[p_global] = outP
    return np.ascontiguousarray(outF), res


def kernel(**inputs) -> np.ndarray:
    out, _ = run(inputs, trace=False)
    return out
